# revision 36
# baseline (speedup 1.0000x reference)
"""Self-contained Trainium2 Bass kernel for nn_Attention (additive attention scores).

kernel(**inputs) takes FULL unsharded inputs and returns the FULL output:
  decoder_hide [32, 512] f32, encoder_out [32, 2048, 1024] f32, mask [32, 2048] i32,
  W_attn [1536, 512] f32, b_attn [512] f32, v_w [512] f32  ->  out [32, 2048] f32

Strategy: data-parallel over batch across 8 NeuronCores (4 batches/core),
weights replicated. Masked positions (~50%) contribute exactly 0 to the
softmax output (exp(-1e5 - max) underflows to 0.0, identically to the
reference), so their encoder rows are never loaded: the kernel gathers only
unmasked rows via indirect DMA (row indices built host-side from the mask,
padded per batch to NTT*128 with duplicate rows that are masked back out
before softmax). This halves HBM traffic, which is the roofline. The
compacted softmax is scattered back to [B, S] on the host during unsharding
(masked positions are exactly 0.0, matching the reference's underflow).

Per core (fp8 pipeline):
  - unmasked encoder rows gathered [s', e] with f32->fp8e4 cast during SWDGE
    indirect DMA (one row index per partition)
  - fp8 PAIRS viewed as fp16 and transposed on the TensorEngine; the pair
    layout is exactly DoubleRow's [K, 2, N] moving-operand access pattern
  - PE proj matmuls in fp8e4 perf_mode=DoubleRow (K=256/instr):
    out[h, s'] += (W_e*256).T @ encT, f32 PSUM; W scaled x256 into fp8 range
  - tanh on ACT with scale=1/256 and per-partition dec_proj bias
  - v-dot on DVE: acc += tanh * v_w[hc] (scalar_tensor_tensor), then one
    ones-diagonal PE matmul per (blk,b) places batch b's logits on partition b
  - pad-mask via copy_predicated; softmax over the compacted axis via
    reduce_max + Exp(bias=-max, accum_out=sum); host scatters the compacted
    softmax back to full [B, S] (masked positions are exactly 0.0)
"""
from contextlib import ExitStack

import numpy as np

B, S, H, E = 32, 2048, 512, 1024
NCORES = 8
B_LOC = B // NCORES
SBLK = 512
WSCALE = 256.0
NPT = 0                 # prefix tiles per batch: rows [0, NPT*128) loaded
                        # wholesale (regular DMA; masked ones discarded by kmask)
NGT = 9                 # max gathered 128-row index tiles per batch
NTT = NPT + NGT
SPAD = NTT * 128        # compacted tokens per batch (>= max unmasked count 1062)
PREF = NPT * 128
BLKS = [(0, 4), (4, 4), (8, NTT - 8)]  # (tile offset, ntiles) per s'-block
# Batches are permuted so that >8-tile ("heavy") batches land in slots 0-1 of
# each core; slots 2-3 statically skip tile 8 (their batches fit in 8 tiles).
SLOT_TILES = (9, 9, 8, 8)

_CACHE = {}


def _build_kernel(repeats=1, stage=4, dbufs=8, pscfg=(3, 2, 2), vwlag=True, enbufs=4,
                  natdt8=True, nogather=False, accbufs=2, vwdepth=1, treng="v"):
    """stage: 1=loads only, 2=+transposes, 3=+proj matmuls, 4=full.
    natdt8=False: load tiles as bf16 (stage-1 diagnostics only).
    nogather=True: replace indirect gathers with regular strided loads of the
    same row count (stage-1 diagnostics only)."""
    import concourse.tile as tile
    from concourse import bacc, mybir, bass as cbass
    from concourse import masks

    F32 = mybir.dt.float32
    BF16 = mybir.dt.bfloat16
    FP16 = mybir.dt.float16
    FP8 = mybir.dt.float8e4
    I32 = mybir.dt.int32
    AF = mybir.ActivationFunctionType
    ALU = mybir.AluOpType
    DR = mybir.MatmulPerfMode.DoubleRow

    NEC2 = E // 256  # number of 256-wide (paired) e-chunks
    NHC = H // 128
    NDC = H // 128

    nc = bacc.Bacc("TRN2", target_bir_lowering=False, debug=False, num_devices=NCORES)

    dec = nc.dram_tensor("decoder_hide", [B_LOC, H], F32, kind="ExternalInput")
    enc = nc.dram_tensor("encoder_out", [B_LOC, S, E], F32, kind="ExternalInput")
    gidx = nc.dram_tensor("gidx", [B_LOC, NGT * 128], I32, kind="ExternalInput")
    kmsk = nc.dram_tensor("kmask", [B_LOC, SPAD], I32, kind="ExternalInput")
    w_attn = nc.dram_tensor("W_attn", [3 * H, H], F32, kind="ExternalInput")
    b_attn = nc.dram_tensor("b_attn", [H], F32, kind="ExternalInput")
    v_w = nc.dram_tensor("v_w", [H], F32, kind="ExternalInput")
    out = nc.dram_tensor("out", [B_LOC, SPAD], F32, kind="ExternalOutput")

    with ExitStack() as ctx:
        tc = ctx.enter_context(tile.TileContext(nc))
        singles = ctx.enter_context(tc.tile_pool(name="singles", bufs=1))
        natp = ctx.enter_context(tc.tile_pool(name="natp", bufs=dbufs))
        trp = ctx.enter_context(tc.tile_pool(name="trp", bufs=dbufs))
        enp = ctx.enter_context(tc.tile_pool(name="enp", bufs=enbufs))
        accp = ctx.enter_context(tc.tile_pool(name="accp", bufs=accbufs))
        psp = ctx.enter_context(tc.tile_pool(name="psp", bufs=pscfg[0], space="PSUM"))
        attp = ctx.enter_context(tc.tile_pool(name="attp", bufs=pscfg[1], space="PSUM"))
        dpp = ctx.enter_context(tc.tile_pool(name="dpp", bufs=1, space="PSUM"))
        trpp = ctx.enter_context(tc.tile_pool(name="trpp", bufs=pscfg[2], space="PSUM"))

        # flat row view for the indirect gather (row index = b*S + s)
        encf = enc[:, :, :].rearrange("b s e -> (b s) e")

        # ---- constants ----
        # W_e pair-gathered: wepair_f[p, ec2, i, h] = W_e[256*ec2 + 2*p + i, h]
        wepair_f = singles.tile([128, NEC2, 2, H], F32)
        nc.sync.dma_start(
            out=wepair_f[:],
            in_=w_attn[H:, :].rearrange("(ec2 p two) h -> p ec2 two h", p=128, two=2),
        )
        we8 = singles.tile([128, NEC2, 2, H], FP8)
        nc.vector.tensor_scalar_mul(we8[:], wepair_f[:], WSCALE)

        wh_f = singles.tile([128, NDC, H], F32)
        nc.sync.dma_start(
            out=wh_f[:], in_=w_attn[:H, :].rearrange("(dc p) h -> p dc h", p=128)
        )
        batt = singles.tile([128, NHC], F32)
        nc.sync.dma_start(out=batt[:], in_=b_attn.rearrange("(hc p) -> p hc", p=128))
        vwb = singles.tile([128, NHC], F32)
        nc.sync.dma_start(out=vwb[:], in_=v_w.rearrange("(hc p) -> p hc", p=128))
        dect = singles.tile([128, NDC, B_LOC], F32)
        for dc in range(NDC):
            nc.gpsimd.dma_start(
                out=dect[:, dc, :],
                in_=dec[:, dc * 128 : (dc + 1) * 128].rearrange("b p -> p b"),
            )
        # gather indices: gidxt[p, b, t] = gidx[b, t*128 + p]
        gidxt = singles.tile([128, B_LOC, NGT], I32)
        nc.sync.dma_start(
            out=gidxt[:], in_=gidx[:, :].rearrange("b (t p) -> p b t", p=128)
        )
        kmaskt = singles.tile([B_LOC, SPAD], I32)
        nc.sync.dma_start(out=kmaskt[:], in_=kmsk[:, :])

        # identity for PE-mode fp16 pair-transposes
        ident = singles.tile([128, 128], FP16)
        masks.make_identity(nc, ident[:])

        # ones-diagonal columns: oz[:, b, :] = [128, B_LOC] bf16, col b = 1.0
        oz = singles.tile([128, B_LOC, B_LOC], BF16)
        nc.vector.memset(oz[:], 0.0)
        for b in range(B_LOC):
            nc.vector.memset(oz[:, b, b : b + 1], 1.0)

        # ---- dec_proj bias: decb[:, hc, b] = W_h.T @ dec.T + b_attn ----
        decb = singles.tile([128, NHC, B_LOC], F32)
        for hc in range(NHC):
            dp = dpp.tile([128, B_LOC], F32)
            for dc in range(NDC):
                nc.tensor.matmul(
                    dp[:],
                    wh_f[:, dc, hc * 128 : (hc + 1) * 128],
                    dect[:, dc, :],
                    start=(dc == 0),
                    stop=(dc == NDC - 1),
                )
            nc.scalar.activation(
                decb[:, hc, :], dp[:], AF.Identity, bias=batt[:, hc : hc + 1]
            )

        Lc = singles.tile([B_LOC, SPAD], F32)

        # ---- main loop over s'-blocks and batches ----
        import contextlib

        loop_ctx = tc.For_i(0, repeats, 1) if repeats > 1 else contextlib.nullcontext()
        with loop_ctx:
          for t0, nt in BLKS:
            N = nt * 128
            c0 = t0 * 128
            # slots whose batch needs tiles in this block
            nb = sum(1 for st in SLOT_TILES if st > t0)
            attps = attp.tile([B_LOC, SBLK], F32)
            # ---- phase 1: gather + transpose + copy for ALL slots of this
            # block, so phase-2 matmuls never wait on a just-issued DVE copy
            trs = []
            for b in range(nb):
                nat = natp.tile([128, 4, E], FP8 if natdt8 else BF16)
                sub = 0
                while sub < nt:
                    t = t0 + sub
                    if nogather:
                        nc.gpsimd.dma_start(
                            out=nat[:, sub : sub + 1, :],
                            in_=enc[
                                b, t * 128 : (t + 1) * 128, :
                            ].rearrange("(sub p) e -> p sub e", p=128),
                        )
                        sub += 1
                    elif t < NPT:
                        # prefix rows: one regular (cheap-descriptor) DMA
                        npre = NPT - t
                        nc.gpsimd.dma_start(
                            out=nat[:, sub : sub + npre, :],
                            in_=enc[
                                b, t * 128 : (t + npre) * 128, :
                            ].rearrange("(sub p) e -> p sub e", p=128),
                        )
                        sub += npre
                    else:
                        nc.gpsimd.indirect_dma_start(
                            out=nat[:, sub, :],
                            out_offset=None,
                            in_=encf,
                            in_offset=cbass.IndirectOffsetOnAxis(
                                ap=gidxt[:, b, t - NPT : t - NPT + 1], axis=0
                            ),
                        )
                        sub += 1
                if stage < 2:
                    nc.vector.tensor_copy(Lc[0:1, c0 + b : c0 + b + 1], nat[0:1, 0, 0:1])
                    trs.append(None)
                    continue
                # fp16 view of fp8 pairs: nat16[p, sub, j] = enc pair (e=2j, 2j+1)
                nat16 = nat[:].bitcast(FP16)
                tr = trp.tile([128, NEC2, SBLK], FP16)
                for ec2 in range(NEC2):
                    trps = trpp.tile([128, SBLK], FP16)
                    for sub in range(nt):
                        nc.tensor.transpose(
                            trps[:, sub * 128 : (sub + 1) * 128],
                            nat16[:, sub, ec2 * 128 : (ec2 + 1) * 128],
                            ident[:],
                        )
                    if treng == "s":
                        nc.scalar.copy(tr[:, ec2, :N], trps[:, :N])
                    else:
                        nc.vector.tensor_copy(tr[:, ec2, :N], trps[:, :N])
                if stage < 3:
                    nc.vector.tensor_copy(Lc[0:1, c0 + b : c0 + b + 1], tr[0:1, 0, 0:1])
                    trs.append(None)
                    continue
                trs.append(tr)
            if stage < 3:
                continue
            # ---- phase 2: proj matmuls + tanh + v-dot accumulate
            pending = []  # lagged v-dot matmuls: (b, acc, N)
            for b in range(nb):
                # fp8 pair view of transposed tile for DoubleRow rhs
                tr8 = trs[b][:].bitcast(FP8)  # [128, NEC2, 2*SBLK]
                acc = accp.tile([128, SBLK], BF16)
                for hc in range(NHC):
                    ps = psp.tile([128, SBLK], F32)
                    for ec2 in range(NEC2):
                        nc.tensor.matmul(
                            ps[:, :N],
                            we8[:, ec2, :, hc * 128 : (hc + 1) * 128],
                            tr8[:, ec2, : 2 * N].rearrange("p (s two) -> p two s", two=2),
                            start=(ec2 == 0),
                            stop=(ec2 == NEC2 - 1),
                            perf_mode=DR,
                        )
                    if stage < 4:
                        nc.vector.tensor_copy(
                            Lc[0:1, c0 + b * NHC + hc : c0 + b * NHC + hc + 1],
                            ps[0:1, 0:1],
                        )
                        continue
                    en = enp.tile([128, SBLK], BF16)
                    nc.scalar.activation(
                        en[:, :N], ps[:, :N], AF.Tanh,
                        bias=decb[:, hc, b : b + 1], scale=1.0 / WSCALE,
                    )
                    if hc == 0:
                        nc.vector.tensor_scalar_mul(acc[:, :N], en[:, :N], vwb[:, 0:1])
                    else:
                        nc.vector.scalar_tensor_tensor(
                            acc[:, :N], en[:, :N], vwb[:, hc : hc + 1], acc[:, :N],
                            ALU.mult, ALU.add,
                        )
                if stage < 4:
                    continue
                if vwlag:
                    # emit the PREVIOUS b's v-dot here so its tanh/DVE chain
                    # had a full projection block to finish
                    pending.append((b, acc, N))
                    if len(pending) > vwdepth:
                        pb, pacc, pN = pending.pop(0)
                        nc.tensor.matmul(
                            attps[:, :pN], oz[:, pb, :], pacc[:, :pN],
                            start=(pb == 0), stop=(pb == nb - 1),
                        )
                else:
                    nc.tensor.matmul(
                        attps[:, :N], oz[:, b, :], acc[:, :N],
                        start=(b == 0), stop=(b == nb - 1),
                    )
            if stage >= 4:
                for pb, pacc, pN in pending:
                    nc.tensor.matmul(
                        attps[:, :pN], oz[:, pb, :], pacc[:, :pN],
                        start=(pb == 0), stop=(pb == nb - 1),
                    )
                nc.vector.tensor_copy(Lc[:nb, c0 : c0 + N], attps[:nb, :N])

        # ---- pad-mask + softmax over gathered axis ----
        Lm = singles.tile([B_LOC, SPAD], F32)
        nc.vector.memset(Lm[:], -100000.0)
        nc.vector.copy_predicated(Lm[:], kmaskt[:], Lc[:])
        M = singles.tile([B_LOC, 1], F32)
        nc.vector.tensor_reduce(
            M[:], Lm[:], axis=mybir.AxisListType.X, op=mybir.AluOpType.max
        )
        negM = singles.tile([B_LOC, 1], F32)
        nc.vector.tensor_scalar_mul(negM[:], M[:], -1.0)
        Ex = singles.tile([B_LOC, SPAD], F32)
        Ssum = singles.tile([B_LOC, 1], F32)
        nc.scalar.activation(
            Ex[:], Lm[:], AF.Exp, bias=negM[:], scale=1.0, accum_out=Ssum[:]
        )
        R = singles.tile([B_LOC, 1], F32)
        nc.vector.reciprocal(R[:], Ssum[:])
        O = singles.tile([B_LOC, SPAD], F32)
        nc.vector.tensor_scalar_mul(O[:], Ex[:], R[:])
        nc.sync.dma_start(out=out[:, :], in_=O[:])

    nc.compile()
    return nc


def _get_state():
    if _CACHE:
        return _CACHE
    import jax
    from jax.experimental.shard_map import shard_map
    from jax.sharding import Mesh, PartitionSpec
    from concourse import bass2jax, mybir

    nc = _build_kernel()
    bass2jax.install_neuronx_cc_hook()

    partition_name = nc.partition_id_tensor.name if nc.partition_id_tensor else None
    in_names: list[str] = []
    out_names: list[str] = []
    out_avals = []
    zero_shapes = []
    for alloc in nc.m.functions[0].allocations:
        if not isinstance(alloc, mybir.MemoryLocationSet):
            continue
        name = alloc.memorylocations[0].name
        if alloc.kind == "ExternalInput":
            if name != partition_name:
                in_names.append(name)
        elif alloc.kind == "ExternalOutput":
            shape = tuple(alloc.tensor_shape)
            dtype = mybir.dt.np(alloc.dtype)
            out_names.append(name)
            out_avals.append(jax.core.ShapedArray(shape, dtype))
            zero_shapes.append((shape, dtype))
    n_params = len(in_names)
    all_names = list(in_names + out_names)
    if partition_name is not None:
        all_names.append(partition_name)
    all_names = tuple(all_names)

    def _body(*args):
        operands = list(args)
        if partition_name is not None:
            operands.append(bass2jax.partition_id_tensor())
        outs = bass2jax._bass_exec_p.bind(
            *operands,
            out_avals=tuple(out_avals),
            in_names=all_names,
            out_names=tuple(out_names),
            lowering_input_output_aliases=(),
            sim_require_finite=True,
            sim_require_nnan=True,
            nc=nc,
        )
        return tuple(outs)

    devices = jax.devices()[:NCORES]
    mesh = Mesh(np.asarray(devices), ("core",))
    n_outs = len(out_names)
    in_specs = (PartitionSpec("core"),) * (n_params + n_outs)
    out_specs = (PartitionSpec("core"),) * n_outs
    donate = tuple(range(n_params, n_params + n_outs))
    fn = jax.jit(
        shard_map(_body, mesh=mesh, in_specs=in_specs, out_specs=out_specs, check_rep=False),
        donate_argnums=donate,
        keep_unused=True,
    )
    _CACHE.update(
        dict(fn=fn, nc=nc, in_names=in_names, out_names=out_names, zero_shapes=zero_shapes, mesh=mesh)
    )
    return _CACHE


def _gather_meta(mask):
    """Compacted layout per batch: [PREF wholesale rows | gathered unmasked
    suffix rows, padded with distinct masked rows]. Batches are permuted so
    heavy (>8-tile) batches occupy slots where SLOT_TILES allows 9 tiles.
    Returns (gidx, kmask, idx_lists, perm) — arrays in PERMUTED order, lists
    indexed by permuted position with the original batch recorded."""
    counts = [int(mask[gb, PREF:].sum()) for gb in range(B)]
    heavy = [gb for gb in range(B) if counts[gb] > 8 * 128]
    light = [gb for gb in range(B) if counts[gb] <= 8 * 128]
    cap = sum(1 for c in range(NCORES) for s, st in enumerate(SLOT_TILES) if st == NGT)
    assert len(heavy) <= cap, f"{len(heavy)} heavy batches exceed capacity {cap}"
    # slot-major positions: all slot-0 positions first, then slot-1, ...
    order = [c * B_LOC + s for s in range(B_LOC) for c in range(NCORES)]
    perm = [0] * B
    for i, gb in enumerate(heavy + light):
        perm[order[i]] = gb
    idx_lists = []
    gidx = np.zeros((B, NGT * 128), np.int32)
    kmask = np.zeros((B, SPAD), np.int32)
    for pos in range(B):
        gb = perm[pos]
        slot = pos % B_LOC
        ntiles = SLOT_TILES[slot]
        sidx = (PREF + np.nonzero(mask[gb, PREF:])[0]).astype(np.int32)
        k = len(sidx)
        assert k <= ntiles * 128, f"batch {gb}: {k} rows exceed slot cap {ntiles * 128}"
        idx_lists.append((gb, sidx))
        # padding entries use DISTINCT masked rows (logits kmask'd out)
        mrows = (PREF + np.nonzero(mask[gb, PREF:] == 0)[0]).astype(np.int32)
        npad = ntiles * 128 - k
        padded = np.zeros(NGT * 128, np.int32)
        padded[:k] = sidx
        padded[k : ntiles * 128] = mrows[:npad]
        gidx[pos] = slot * S + padded
        kmask[pos, :PREF] = mask[gb, :PREF]
        kmask[pos, PREF : PREF + k] = 1
    return gidx, kmask, idx_lists, perm


def _concat_inputs(inputs):
    """Build the global (concat over cores on axis 0) arrays in in_names order."""
    st = _get_state()
    gidx, kmask, _, perm = _gather_meta(np.asarray(inputs["mask"]))
    per_name = {}
    per_name["decoder_hide"] = np.asarray(inputs["decoder_hide"])[perm]
    per_name["encoder_out"] = np.asarray(inputs["encoder_out"])[perm]
    per_name["gidx"] = gidx
    per_name["kmask"] = kmask
    # replicated weights: tile along axis 0
    per_name["W_attn"] = np.tile(inputs["W_attn"], (NCORES, 1))
    per_name["b_attn"] = np.tile(inputs["b_attn"], NCORES)
    per_name["v_w"] = np.tile(inputs["v_w"], NCORES)
    return [np.ascontiguousarray(per_name[n]) for n in st["in_names"]]


def _zero_outs():
    st = _get_state()
    return [
        np.zeros((NCORES * shape[0], *shape[1:]), dtype) for shape, dtype in st["zero_shapes"]
    ]


def kernel(**inputs) -> np.ndarray:
    st = _get_state()
    concat_in = _concat_inputs(inputs)
    outs = st["fn"](*concat_in, *_zero_outs())
    comp = np.asarray(outs[st["out_names"].index("out")]).reshape(B, SPAD)
    _, _, idx_lists, _ = _gather_meta(np.asarray(inputs["mask"]))
    res = np.zeros((B, S), np.float32)
    for pos in range(B):
        # prefix positions map 1:1 (masked ones are exactly 0.0, as in the
        # reference); gathered suffix positions scatter by index
        gb, sidx = idx_lists[pos]
        res[gb, :PREF] = comp[pos, :PREF]
        res[gb, sidx] = comp[pos, PREF : PREF + len(sidx)]
    return res


# --- helpers for test.py timing ---
def run_on_device(dev_in, dev_zeros):
    st = _get_state()
    return st["fn"](*dev_in, *dev_zeros)


def last_exec_estimate_ns():
    return _CACHE.get("exec_ns", None)


# revision 40
# speedup vs baseline: 1.0372x; 1.0372x over previous
"""Self-contained Trainium2 Bass kernel for nn_Attention (additive attention scores).

kernel(**inputs) takes FULL unsharded inputs and returns the FULL output:
  decoder_hide [32, 512] f32, encoder_out [32, 2048, 1024] f32, mask [32, 2048] i32,
  W_attn [1536, 512] f32, b_attn [512] f32, v_w [512] f32  ->  out [32, 2048] f32

Strategy: data-parallel over batch across 8 NeuronCores (4 batches/core),
weights replicated. Masked positions (~50%) contribute exactly 0 to the
softmax output (exp(-1e5 - max) underflows to 0.0, identically to the
reference), so their encoder rows are never loaded: the kernel gathers only
unmasked rows via indirect DMA (row indices built host-side from the mask,
padded per batch to NTT*128 with duplicate rows that are masked back out
before softmax). This halves HBM traffic, which is the roofline. The
compacted softmax is scattered back to [B, S] on the host during unsharding
(masked positions are exactly 0.0, matching the reference's underflow).

Per core (fp8 pipeline):
  - unmasked encoder rows gathered [s', e] with f32->fp8e4 cast during SWDGE
    indirect DMA (one row index per partition)
  - fp8 PAIRS viewed as fp16 and transposed on the TensorEngine; the pair
    layout is exactly DoubleRow's [K, 2, N] moving-operand access pattern
  - PE proj matmuls in fp8e4 perf_mode=DoubleRow (K=256/instr):
    out[h, s'] += (W_e*256).T @ encT, f32 PSUM; W scaled x256 into fp8 range
  - tanh on ACT with scale=1/256 and per-partition dec_proj bias
  - v-dot on DVE: acc += tanh * v_w[hc] (scalar_tensor_tensor), then one
    ones-diagonal PE matmul per (blk,b) places batch b's logits on partition b
  - pad-mask via copy_predicated; softmax over the compacted axis via
    reduce_max + Exp(bias=-max, accum_out=sum); host scatters the compacted
    softmax back to full [B, S] (masked positions are exactly 0.0)
"""
from contextlib import ExitStack

import numpy as np

B, S, H, E = 32, 2048, 512, 1024
NCORES = 8
B_LOC = B // NCORES
SBLK = 512
WSCALE = 256.0
NPT = 0                 # prefix tiles per batch: rows [0, NPT*128) loaded
                        # wholesale (regular DMA; masked ones discarded by kmask)
NGT = 9                 # max gathered 128-row index tiles per batch
NTT = NPT + NGT
SPAD = NTT * 128        # compacted tokens per batch (>= max unmasked count 1062)
PREF = NPT * 128
BLKS = [(0, 4), (4, 4), (8, NTT - 8)]  # (tile offset, ntiles) per s'-block
# Batches are permuted so that >8-tile ("heavy") batches land in slots 0-1 of
# each core; slots 2-3 statically skip tile 8 (their batches fit in 8 tiles).
SLOT_TILES = (9, 9, 8, 8)

_CACHE = {}


def _build_kernel(repeats=1, stage=4, dbufs=8, pscfg=(3, 2, 2), vwlag=True, enbufs=4,
                  natdt8=True, nogather=False, accbufs=2, vwdepth=1, treng="v"):
    """stage: 1=loads only, 2=+transposes, 3=+proj matmuls, 4=full.
    natdt8=False: load tiles as bf16 (stage-1 diagnostics only).
    nogather=True: replace indirect gathers with regular strided loads of the
    same row count (stage-1 diagnostics only)."""
    import concourse.tile as tile
    from concourse import bacc, mybir, bass as cbass
    from concourse import masks

    F32 = mybir.dt.float32
    BF16 = mybir.dt.bfloat16
    FP16 = mybir.dt.float16
    FP8 = mybir.dt.float8e4
    I32 = mybir.dt.int32
    AF = mybir.ActivationFunctionType
    ALU = mybir.AluOpType
    DR = mybir.MatmulPerfMode.DoubleRow

    NEC2 = E // 256  # number of 256-wide (paired) e-chunks
    NHC = H // 128
    NDC = H // 128

    nc = bacc.Bacc("TRN2", target_bir_lowering=False, debug=False, num_devices=NCORES)

    dec = nc.dram_tensor("decoder_hide", [B_LOC, H], F32, kind="ExternalInput")
    enc = nc.dram_tensor("encoder_out", [B_LOC, S, E], F32, kind="ExternalInput")
    gidx = nc.dram_tensor("gidx", [B_LOC, NGT * 128], I32, kind="ExternalInput")
    kmsk = nc.dram_tensor("kmask", [B_LOC, SPAD], I32, kind="ExternalInput")
    w_attn = nc.dram_tensor("W_attn", [3 * H, H], F32, kind="ExternalInput")
    b_attn = nc.dram_tensor("b_attn", [H], F32, kind="ExternalInput")
    v_w = nc.dram_tensor("v_w", [H], F32, kind="ExternalInput")
    out = nc.dram_tensor("out", [B_LOC, SPAD], F32, kind="ExternalOutput")

    with ExitStack() as ctx:
        tc = ctx.enter_context(tile.TileContext(nc))
        singles = ctx.enter_context(tc.tile_pool(name="singles", bufs=1))
        natp = ctx.enter_context(tc.tile_pool(name="natp", bufs=dbufs))
        trp = ctx.enter_context(tc.tile_pool(name="trp", bufs=dbufs))
        enp = ctx.enter_context(tc.tile_pool(name="enp", bufs=enbufs))
        accp = ctx.enter_context(tc.tile_pool(name="accp", bufs=accbufs))
        psp = ctx.enter_context(tc.tile_pool(name="psp", bufs=pscfg[0], space="PSUM"))
        attp = ctx.enter_context(tc.tile_pool(name="attp", bufs=pscfg[1], space="PSUM"))
        dpp = ctx.enter_context(tc.tile_pool(name="dpp", bufs=1, space="PSUM"))
        trpp = ctx.enter_context(tc.tile_pool(name="trpp", bufs=pscfg[2], space="PSUM"))

        # flat row view for the indirect gather (row index = b*S + s)
        encf = enc[:, :, :].rearrange("b s e -> (b s) e")

        # ---- constants ----
        # W_e pair-gathered: wepair_f[p, ec2, i, h] = W_e[256*ec2 + 2*p + i, h]
        wepair_f = singles.tile([128, NEC2, 2, H], F32)
        nc.sync.dma_start(
            out=wepair_f[:],
            in_=w_attn[H:, :].rearrange("(ec2 p two) h -> p ec2 two h", p=128, two=2),
        )
        we8 = singles.tile([128, NEC2, 2, H], FP8)
        nc.vector.tensor_scalar_mul(we8[:], wepair_f[:], WSCALE)

        wh_f = singles.tile([128, NDC, H], F32)
        nc.sync.dma_start(
            out=wh_f[:], in_=w_attn[:H, :].rearrange("(dc p) h -> p dc h", p=128)
        )
        batt = singles.tile([128, NHC], F32)
        nc.sync.dma_start(out=batt[:], in_=b_attn.rearrange("(hc p) -> p hc", p=128))
        vwb = singles.tile([128, NHC], F32)
        nc.sync.dma_start(out=vwb[:], in_=v_w.rearrange("(hc p) -> p hc", p=128))
        dect = singles.tile([128, NDC, B_LOC], F32)
        for dc in range(NDC):
            nc.gpsimd.dma_start(
                out=dect[:, dc, :],
                in_=dec[:, dc * 128 : (dc + 1) * 128].rearrange("b p -> p b"),
            )
        # gather indices: gidxt[p, b, t] = gidx[b, t*128 + p]
        gidxt = singles.tile([128, B_LOC, NGT], I32)
        nc.sync.dma_start(
            out=gidxt[:], in_=gidx[:, :].rearrange("b (t p) -> p b t", p=128)
        )
        kmaskt = singles.tile([B_LOC, SPAD], I32)
        nc.sync.dma_start(out=kmaskt[:], in_=kmsk[:, :])

        # identity for PE-mode fp16 pair-transposes
        ident = singles.tile([128, 128], FP16)
        masks.make_identity(nc, ident[:])

        # ones-diagonal columns: oz[:, b, :] = [128, B_LOC] bf16, col b = 1.0
        oz = singles.tile([128, B_LOC, B_LOC], BF16)
        nc.vector.memset(oz[:], 0.0)
        for b in range(B_LOC):
            nc.vector.memset(oz[:, b, b : b + 1], 1.0)

        # ---- dec_proj bias: decb[:, hc, b] = W_h.T @ dec.T + b_attn ----
        decb = singles.tile([128, NHC, B_LOC], F32)
        for hc in range(NHC):
            dp = dpp.tile([128, B_LOC], F32)
            for dc in range(NDC):
                nc.tensor.matmul(
                    dp[:],
                    wh_f[:, dc, hc * 128 : (hc + 1) * 128],
                    dect[:, dc, :],
                    start=(dc == 0),
                    stop=(dc == NDC - 1),
                )
            nc.scalar.activation(
                decb[:, hc, :], dp[:], AF.Identity, bias=batt[:, hc : hc + 1]
            )

        Lc = singles.tile([B_LOC, SPAD], F32)

        # ---- main loop over s'-blocks and batches ----
        import contextlib

        loop_ctx = tc.For_i(0, repeats, 1) if repeats > 1 else contextlib.nullcontext()
        with loop_ctx:
          for t0, nt in BLKS:
            N = nt * 128
            c0 = t0 * 128
            # slots whose batch needs tiles in this block
            nb = sum(1 for st in SLOT_TILES if st > t0)
            attps = attp.tile([B_LOC, SBLK], F32)
            pending = []  # lagged v-dot matmuls: (b, acc, N)
            for b in range(nb):
                nat = natp.tile([128, 4, E], FP8 if natdt8 else BF16)
                sub = 0
                while sub < nt:
                    t = t0 + sub
                    if nogather:
                        nc.gpsimd.dma_start(
                            out=nat[:, sub : sub + 1, :],
                            in_=enc[
                                b, t * 128 : (t + 1) * 128, :
                            ].rearrange("(sub p) e -> p sub e", p=128),
                        )
                        sub += 1
                    elif t < NPT:
                        # prefix rows: one regular (cheap-descriptor) DMA
                        npre = NPT - t
                        nc.gpsimd.dma_start(
                            out=nat[:, sub : sub + npre, :],
                            in_=enc[
                                b, t * 128 : (t + npre) * 128, :
                            ].rearrange("(sub p) e -> p sub e", p=128),
                        )
                        sub += npre
                    else:
                        nc.gpsimd.indirect_dma_start(
                            out=nat[:, sub, :],
                            out_offset=None,
                            in_=encf,
                            in_offset=cbass.IndirectOffsetOnAxis(
                                ap=gidxt[:, b, t - NPT : t - NPT + 1], axis=0
                            ),
                        )
                        sub += 1
                if stage < 2:
                    nc.vector.tensor_copy(Lc[0:1, c0 + b : c0 + b + 1], nat[0:1, 0, 0:1])
                    continue
                # fp16 view of fp8 pairs: nat16[p, sub, j] = enc pair (e=2j, 2j+1)
                nat16 = nat[:].bitcast(FP16)
                tr = trp.tile([128, NEC2, SBLK], FP16)
                for ec2 in range(NEC2):
                    trps = trpp.tile([128, SBLK], FP16)
                    for sub in range(nt):
                        nc.tensor.transpose(
                            trps[:, sub * 128 : (sub + 1) * 128],
                            nat16[:, sub, ec2 * 128 : (ec2 + 1) * 128],
                            ident[:],
                        )
                    if treng == "s":
                        nc.scalar.copy(tr[:, ec2, :N], trps[:, :N])
                    else:
                        nc.vector.tensor_copy(tr[:, ec2, :N], trps[:, :N])
                if stage < 3:
                    nc.vector.tensor_copy(Lc[0:1, c0 + b : c0 + b + 1], tr[0:1, 0, 0:1])
                    continue
                # fp8 pair view of transposed tile for DoubleRow rhs
                tr8 = tr[:].bitcast(FP8)  # [128, NEC2, 2*SBLK]
                acc = accp.tile([128, SBLK], BF16)
                for hc in range(NHC):
                    ps = psp.tile([128, SBLK], F32)
                    for ec2 in range(NEC2):
                        nc.tensor.matmul(
                            ps[:, :N],
                            we8[:, ec2, :, hc * 128 : (hc + 1) * 128],
                            tr8[:, ec2, : 2 * N].rearrange("p (s two) -> p two s", two=2),
                            start=(ec2 == 0),
                            stop=(ec2 == NEC2 - 1),
                            perf_mode=DR,
                        )
                    if stage < 4:
                        nc.vector.tensor_copy(
                            Lc[0:1, c0 + b * NHC + hc : c0 + b * NHC + hc + 1],
                            ps[0:1, 0:1],
                        )
                        continue
                    en = enp.tile([128, SBLK], BF16)
                    nc.scalar.activation(
                        en[:, :N], ps[:, :N], AF.Tanh,
                        bias=decb[:, hc, b : b + 1], scale=1.0 / WSCALE,
                    )
                    if hc == 0:
                        nc.vector.tensor_scalar_mul(acc[:, :N], en[:, :N], vwb[:, 0:1])
                    else:
                        nc.vector.scalar_tensor_tensor(
                            acc[:, :N], en[:, :N], vwb[:, hc : hc + 1], acc[:, :N],
                            ALU.mult, ALU.add,
                        )
                if stage < 4:
                    continue
                if vwlag:
                    # emit the PREVIOUS b's v-dot here so its tanh/DVE chain
                    # had a full projection block to finish
                    pending.append((b, acc, N))
                    if len(pending) > vwdepth:
                        pb, pacc, pN = pending.pop(0)
                        nc.tensor.matmul(
                            attps[:, :pN], oz[:, pb, :], pacc[:, :pN],
                            start=(pb == 0), stop=(pb == nb - 1),
                        )
                else:
                    nc.tensor.matmul(
                        attps[:, :N], oz[:, b, :], acc[:, :N],
                        start=(b == 0), stop=(b == nb - 1),
                    )
            if stage >= 4:
                for pb, pacc, pN in pending:
                    nc.tensor.matmul(
                        attps[:, :pN], oz[:, pb, :], pacc[:, :pN],
                        start=(pb == 0), stop=(pb == nb - 1),
                    )
                nc.vector.tensor_copy(Lc[:nb, c0 : c0 + N], attps[:nb, :N])

        # ---- pad-mask + softmax over gathered axis ----
        Lm = singles.tile([B_LOC, SPAD], F32)
        nc.vector.memset(Lm[:], -100000.0)
        nc.vector.copy_predicated(Lm[:], kmaskt[:], Lc[:])
        M = singles.tile([B_LOC, 1], F32)
        nc.vector.tensor_reduce(
            M[:], Lm[:], axis=mybir.AxisListType.X, op=mybir.AluOpType.max
        )
        negM = singles.tile([B_LOC, 1], F32)
        nc.vector.tensor_scalar_mul(negM[:], M[:], -1.0)
        Ex = singles.tile([B_LOC, SPAD], F32)
        Ssum = singles.tile([B_LOC, 1], F32)
        nc.scalar.activation(
            Ex[:], Lm[:], AF.Exp, bias=negM[:], scale=1.0, accum_out=Ssum[:]
        )
        R = singles.tile([B_LOC, 1], F32)
        nc.vector.reciprocal(R[:], Ssum[:])
        O = singles.tile([B_LOC, SPAD], F32)
        nc.vector.tensor_scalar_mul(O[:], Ex[:], R[:])
        nc.sync.dma_start(out=out[:, :], in_=O[:])

    nc.compile()
    return nc


def _get_state():
    if _CACHE:
        return _CACHE
    import jax
    from jax.experimental.shard_map import shard_map
    from jax.sharding import Mesh, PartitionSpec
    from concourse import bass2jax, mybir

    nc = _build_kernel()
    bass2jax.install_neuronx_cc_hook()

    partition_name = nc.partition_id_tensor.name if nc.partition_id_tensor else None
    in_names: list[str] = []
    out_names: list[str] = []
    out_avals = []
    zero_shapes = []
    for alloc in nc.m.functions[0].allocations:
        if not isinstance(alloc, mybir.MemoryLocationSet):
            continue
        name = alloc.memorylocations[0].name
        if alloc.kind == "ExternalInput":
            if name != partition_name:
                in_names.append(name)
        elif alloc.kind == "ExternalOutput":
            shape = tuple(alloc.tensor_shape)
            dtype = mybir.dt.np(alloc.dtype)
            out_names.append(name)
            out_avals.append(jax.core.ShapedArray(shape, dtype))
            zero_shapes.append((shape, dtype))
    n_params = len(in_names)
    all_names = list(in_names + out_names)
    if partition_name is not None:
        all_names.append(partition_name)
    all_names = tuple(all_names)

    def _body(*args):
        operands = list(args)
        if partition_name is not None:
            operands.append(bass2jax.partition_id_tensor())
        outs = bass2jax._bass_exec_p.bind(
            *operands,
            out_avals=tuple(out_avals),
            in_names=all_names,
            out_names=tuple(out_names),
            lowering_input_output_aliases=(),
            sim_require_finite=True,
            sim_require_nnan=True,
            nc=nc,
        )
        return tuple(outs)

    devices = jax.devices()[:NCORES]
    mesh = Mesh(np.asarray(devices), ("core",))
    n_outs = len(out_names)
    in_specs = (PartitionSpec("core"),) * (n_params + n_outs)
    out_specs = (PartitionSpec("core"),) * n_outs
    donate = tuple(range(n_params, n_params + n_outs))
    fn = jax.jit(
        shard_map(_body, mesh=mesh, in_specs=in_specs, out_specs=out_specs, check_rep=False),
        donate_argnums=donate,
        keep_unused=True,
    )
    _CACHE.update(
        dict(fn=fn, nc=nc, in_names=in_names, out_names=out_names, zero_shapes=zero_shapes, mesh=mesh)
    )
    return _CACHE


def _gather_meta(mask):
    """Compacted layout per batch: [PREF wholesale rows | gathered unmasked
    suffix rows, padded with distinct masked rows]. Batches are permuted so
    heavy (>8-tile) batches occupy slots where SLOT_TILES allows 9 tiles.
    Returns (gidx, kmask, idx_lists, perm) — arrays in PERMUTED order, lists
    indexed by permuted position with the original batch recorded."""
    counts = [int(mask[gb, PREF:].sum()) for gb in range(B)]
    heavy = [gb for gb in range(B) if counts[gb] > 8 * 128]
    light = [gb for gb in range(B) if counts[gb] <= 8 * 128]
    cap = sum(1 for c in range(NCORES) for s, st in enumerate(SLOT_TILES) if st == NGT)
    assert len(heavy) <= cap, f"{len(heavy)} heavy batches exceed capacity {cap}"
    # slot-major positions: all slot-0 positions first, then slot-1, ...
    order = [c * B_LOC + s for s in range(B_LOC) for c in range(NCORES)]
    perm = [0] * B
    for i, gb in enumerate(heavy + light):
        perm[order[i]] = gb
    idx_lists = []
    gidx = np.zeros((B, NGT * 128), np.int32)
    kmask = np.zeros((B, SPAD), np.int32)
    for pos in range(B):
        gb = perm[pos]
        slot = pos % B_LOC
        ntiles = SLOT_TILES[slot]
        sidx = (PREF + np.nonzero(mask[gb, PREF:])[0]).astype(np.int32)
        k = len(sidx)
        assert k <= ntiles * 128, f"batch {gb}: {k} rows exceed slot cap {ntiles * 128}"
        idx_lists.append((gb, sidx))
        # padding entries use DISTINCT masked rows (logits kmask'd out)
        mrows = (PREF + np.nonzero(mask[gb, PREF:] == 0)[0]).astype(np.int32)
        npad = ntiles * 128 - k
        padded = np.zeros(NGT * 128, np.int32)
        padded[:k] = sidx
        padded[k : ntiles * 128] = mrows[:npad]
        gidx[pos] = slot * S + padded
        kmask[pos, :PREF] = mask[gb, :PREF]
        kmask[pos, PREF : PREF + k] = 1
    return gidx, kmask, idx_lists, perm


def _concat_inputs(inputs):
    """Build the global (concat over cores on axis 0) arrays in in_names order."""
    st = _get_state()
    gidx, kmask, _, perm = _gather_meta(np.asarray(inputs["mask"]))
    per_name = {}
    per_name["decoder_hide"] = np.asarray(inputs["decoder_hide"])[perm]
    per_name["encoder_out"] = np.asarray(inputs["encoder_out"])[perm]
    per_name["gidx"] = gidx
    per_name["kmask"] = kmask
    # replicated weights: tile along axis 0
    per_name["W_attn"] = np.tile(inputs["W_attn"], (NCORES, 1))
    per_name["b_attn"] = np.tile(inputs["b_attn"], NCORES)
    per_name["v_w"] = np.tile(inputs["v_w"], NCORES)
    return [np.ascontiguousarray(per_name[n]) for n in st["in_names"]]


def _zero_outs():
    st = _get_state()
    return [
        np.zeros((NCORES * shape[0], *shape[1:]), dtype) for shape, dtype in st["zero_shapes"]
    ]


def kernel(**inputs) -> np.ndarray:
    st = _get_state()
    concat_in = _concat_inputs(inputs)
    outs = st["fn"](*concat_in, *_zero_outs())
    comp = np.asarray(outs[st["out_names"].index("out")]).reshape(B, SPAD)
    _, _, idx_lists, _ = _gather_meta(np.asarray(inputs["mask"]))
    res = np.zeros((B, S), np.float32)
    for pos in range(B):
        # prefix positions map 1:1 (masked ones are exactly 0.0, as in the
        # reference); gathered suffix positions scatter by index
        gb, sidx = idx_lists[pos]
        res[gb, :PREF] = comp[pos, :PREF]
        res[gb, sidx] = comp[pos, PREF : PREF + len(sidx)]
    return res


# --- helpers for test.py timing ---
def run_on_device(dev_in, dev_zeros):
    st = _get_state()
    return st["fn"](*dev_in, *dev_zeros)


def last_exec_estimate_ns():
    return _CACHE.get("exec_ns", None)
